# revision 10
# baseline (speedup 1.0000x reference)
"""Trainium2 Bass kernel for dual-branch (causal + anticausal) attention + residual + LayerNorm.

Reference computation (per batch b):
  out_c  = causal_attn(x_b; Wqkv_c, Wp_c)      (mask j <= i)
  out_ac = anticausal_attn(x_b; Wqkv_ac, Wp_ac) (mask j >= i)
  y = LayerNorm(x + out_c + out_ac) * gamma + beta

Sharding: 8 cores = 4 batches x 2 sequence-halves. Each core computes BOTH
branches for its 512 own tokens (recomputing k/v projections for the needed
part of the 1024-token sequence locally -> zero cross-core communication).
A single SPMD program always "owns" the SECOND half of the sequence; cores
responsible for the first half receive the token-REVERSED sequence with the
causal/anticausal weights swapped, and their output rows are un-reversed on
the host.

v2 performance structure:
 - All projection matmuls (q/k, v, out) run in fp8e4m3 with
   perf_mode=DoubleRow (K=256 per matmul -> half the PE stream time).
   Weights are host-scaled by 16 to dodge fp8 subnormals; the 16x factor
   rides through the residual sum and cancels in LayerNorm (scale-invariant).
 - Attention scores run as row-tiled head PAIRS (two K=64 matmuls
   concurrently in the PE array, partitions 0:64 / 64:128).
 - Softmax denominator comes from an appended ones(=16)-column on V;
   diagonal-block masking is done in-place on the exp tiles by GpSimd
   affine_select (no mask tensors, no DVE multiplies).
 - Emission order interleaves branch-1 projections (PE-dense, Scalar-free)
   into branch-0's attention (Scalar-bound), and branch-0's out-projection
   into branch-1's attention, so the PE never starves while ACT does EXPs.
"""

import numpy as np
from contextlib import ExitStack

import concourse.bass as bass
import concourse.tile as tile
import concourse.mybir as mybir
from concourse import bacc
from concourse import bass_utils

F32 = mybir.dt.float32
F16 = mybir.dt.float16
F8 = mybir.dt.float8e4
AF = mybir.ActivationFunctionType
ALU = mybir.AluOpType
DR = mybir.MatmulPerfMode.DoubleRow

DIM = 768
HEADS = 12
HD = 64
T = 1024
OWN = 512
B = 4
EPS = 1e-5
P = 128
CB = DIM // P          # 6 contraction blocks
C2 = CB // 2           # 3 DoubleRow contraction super-blocks (256 each)
TT = T // P            # 8 token tiles (full sequence)
OT = OWN // P          # 4 own token tiles
OWN_CH0 = TT - OT      # own q-chunks are global chunks 4..7
NP = HEADS // 2        # 6 head pairs
WS = 16.0              # host-side weight scale (fp8 subnormal dodge)
EXP_SCALE = 1.0 / (8.0 * WS * WS)   # 1/sqrt(HD) / WS^2

F8NP = mybir.dt.np(F8)


def _f32(x):
    return np.ascontiguousarray(np.asarray(x, dtype=np.float32))


def _f16(x):
    return np.ascontiguousarray(np.asarray(x, dtype=np.float16))


def _f8(x):
    return np.ascontiguousarray(np.asarray(x, dtype=np.float32).astype(F8NP))


def build_program(has_bqkv: bool, has_bp: bool, has_gb: bool):
    nc = bacc.Bacc("TRN2", target_bir_lowering=False)

    xT_d = nc.dram_tensor("xT8", [P, CB, T], F8, kind="ExternalInput")
    xo_d = nc.dram_tensor("x_own", [OWN, DIM], F16, kind="ExternalInput")
    wqk_d = [nc.dram_tensor(f"wqk{i}", [P, 2 * CB, CB, P], F8, kind="ExternalInput") for i in (1, 2)]
    wv_d = [nc.dram_tensor(f"wv{i}", [P, CB, DIM], F8, kind="ExternalInput") for i in (1, 2)]
    wp_d = [nc.dram_tensor(f"wp{i}", [P, CB, DIM], F8, kind="ExternalInput") for i in (1, 2)]
    b_d = [nc.dram_tensor(f"b{i}", [3 * DIM], F32, kind="ExternalInput") for i in (1, 2)]
    bp_d = [nc.dram_tensor(f"bp{i}", [DIM], F32, kind="ExternalInput") for i in (1, 2)]
    gamma_d = nc.dram_tensor("gamma", [DIM], F32, kind="ExternalInput")
    beta_d = nc.dram_tensor("beta", [DIM], F32, kind="ExternalInput")
    y_d = nc.dram_tensor("y", [OWN, DIM], F32, kind="ExternalOutput")

    with tile.TileContext(nc) as tc, ExitStack() as ctx:
        const = ctx.enter_context(tc.tile_pool(name="const", bufs=1))
        persist = ctx.enter_context(tc.tile_pool(name="persist", bufs=1))
        wvp_pool = ctx.enter_context(tc.tile_pool(name="wvp", bufs=1))
        expT_pool = ctx.enter_context(tc.tile_pool(name="expT", bufs=1))
        rb_pool = ctx.enter_context(tc.tile_pool(name="rb", bufs=3))
        stat_pool = ctx.enter_context(tc.tile_pool(name="stat", bufs=8))
        xc_pool = ctx.enter_context(tc.tile_pool(name="xc", bufs=2))
        yacc_pool = ctx.enter_context(tc.tile_pool(name="yacc", bufs=1))
        ps = ctx.enter_context(tc.tile_pool(name="ps", bufs=1, space="PSUM"))

        # ---- constants / full-kernel-lifetime tensors ----
        # xT in fp8, split into 3 DoubleRow contraction super-blocks so the
        # first projection only waits on one third of the input DMA.
        xT8 = [const.tile([P, 2, T], F8, tag=f"xT{c}", name=f"xT{c}") for c in range(C2)]
        # own-half columns first (the q-projection needs only those), spread
        # across the scalar+gpsimd queues; sync is reserved for the wqk stream
        nc.scalar.dma_start(xT8[0][:, :, OWN:T], xT_d[:, 0:2, OWN:T])
        nc.gpsimd.dma_start(xT8[1][:, :, OWN:T], xT_d[:, 2:4, OWN:T])
        nc.scalar.dma_start(xT8[2][:, :, OWN:T], xT_d[:, 4:6, OWN:T])
        nc.gpsimd.dma_start(xT8[0][:, :, 0:OWN], xT_d[:, 0:2, 0:OWN])
        nc.scalar.dma_start(xT8[1][:, :, 0:OWN], xT_d[:, 2:4, 0:OWN])
        nc.gpsimd.dma_start(xT8[2][:, :, 0:OWN], xT_d[:, 4:6, 0:OWN])

        xo_sb = const.tile([P, OT, DIM], F16)
        nc.gpsimd.dma_start(xo_sb[:], xo_d.rearrange("(tb p) c -> p tb c", p=P))

        gamma_b = beta_b = None
        if has_gb:
            gamma_b = const.tile([P, DIM], F32)
            nc.gpsimd.dma_start(gamma_b[:], bass.AP(tensor=gamma_d, offset=0, ap=[[0, P], [1, DIM]]))
            beta_b = const.tile([P, DIM], F32)
            nc.gpsimd.dma_start(beta_b[:], bass.AP(tensor=beta_d, offset=0, ap=[[0, P], [1, DIM]]))

        zbias = const.tile([P, 1], F32)
        nc.vector.memset(zbias[:], 0.0)
        ebias = const.tile([P, 1], F32)
        nc.vector.memset(ebias[:], EPS)

        bp_b = None
        if has_bp:
            bp_b = [const.tile([P, DIM], F32, tag=f"bp_b{i}", name=f"bp_b{i}") for i in range(2)]
            for i in range(2):
                nc.gpsimd.dma_start(bp_b[i][:], bass.AP(tensor=bp_d[i], offset=0, ap=[[0, P], [1, DIM]]))

        # weight tensors (fp8, resident per branch); wqk is loaded as ONE
        # 9.2KB/partition DMA -- per-n-block loads had 768B lines (hundreds
        # of tiny descriptors, ~4.3us each) and starved the PE just-in-time
        wqk_sb = [wvp_pool.tile([P, 2 * CB, CB, P], F8, tag="wqk", name=f"wqk{br}", bufs=2) for br in range(2)]
        nc.sync.dma_start(wqk_sb[0][:], wqk_d[0][:])
        nc.sync.dma_start(wqk_sb[1][:], wqk_d[1][:])
        wv_sb = [wvp_pool.tile([P, CB, DIM], F8, tag="wv", name=f"wv{br}", bufs=2) for br in range(2)]
        wp_sb = [wvp_pool.tile([P, CB, DIM], F8, tag="wp", name=f"wp{br}", bufs=2) for br in range(2)]
        nc.scalar.dma_start(wv_sb[0][:], wv_d[0][:])
        nc.gpsimd.dma_start(wv_sb[1][:], wv_d[1][:])
        nc.scalar.dma_start(wp_sb[0][:], wp_d[0][:])
        nc.gpsimd.dma_start(wp_sb[1][:], wp_d[1][:])

        # y accumulator tiles (live across both branches)
        ys = [yacc_pool.tile([P, DIM], F32, tag=f"ys{t}", name=f"ys{t}") for t in range(OT)]
        y_out = y_d.rearrange("(tb p) c -> tb p c", p=P)

        # ---- per-branch state ----
        class St:
            pass

        sts = []
        for br in range(2):
            st = St()
            st.br = br
            st.causal = br == 0
            st.full_kv = st.causal
            st.kT = [persist.tile([P, T], F16, tag=f"kT{i}", name=f"kT{i}_{br}", bufs=2) for i in range(CB)]
            st.qT = [persist.tile([P, OWN], F16, tag=f"qT{i}", name=f"qT{i}_{br}", bufs=2) for i in range(CB)]
            st.vaug = [persist.tile([P, HEADS * (HD + 1)], F16, tag=f"va{t}", name=f"va{t}_{br}", bufs=2) for t in range(TT)]
            st.oT8 = persist.tile([P, CB, OWN], F8, tag="oT8", name=f"oT8_{br}", bufs=2)
            st.bqk = None
            st.bv = None
            if has_bqkv:
                st.bqk = persist.tile([P, 2 * CB], F32, tag="bqk", bufs=2)
                nc.sync.dma_start(st.bqk[:], b_d[br][0:2 * DIM].rearrange("(n p) -> p n", p=P))
                st.bv = persist.tile([P, DIM], F32, tag="bv", bufs=2)
                nc.sync.dma_start(st.bv[:], bass.AP(tensor=b_d[br], offset=2 * DIM, ap=[[0, P], [1, DIM]]))
            sts.append(st)

        emitted = set()

        # ---- projection steps ----
        def qk_nblock(br, n):
            """q/k projection for one 128-wide output block n (fp8 DoubleRow)."""
            key = ("qk", br, n)
            if key in emitted:
                return
            emitted.add(key)
            st = sts[br]
            is_q = n < CB
            if is_q:
                chunks = [(OWN, OWN)]
            elif st.full_kv:
                chunks = [(0, 512), (512, 512)]
            else:
                chunks = [(512, 512)]
            for (t0, tw) in chunks:
                pst = ps.tile([P, 512], F32, tag="proj", name="proj", bufs=2)
                for c in range(C2):
                    nc.tensor.matmul(
                        pst[:, :tw],
                        wqk_sb[br][:, n, 2 * c:2 * c + 2, :],
                        xT8[c][:, :, t0:t0 + tw],
                        start=(c == 0), stop=(c == C2 - 1),
                        perf_mode=DR,
                    )
                if is_q:
                    dest = st.qT[n][:, :]
                else:
                    dest = st.kT[n - CB][:, t0:t0 + tw]
                if has_bqkv:
                    nc.vector.tensor_scalar_add(dest, pst[:, :tw], st.bqk[:, n:n + 1])
                else:
                    nc.vector.tensor_copy(dest, pst[:, :tw])

        def ensure_qk_pair(br, m):
            qk_nblock(br, m)
            qk_nblock(br, CB + m)

        def v_tile(br, t):
            """v projection for token tile t -> vaug[t] (fp8 DoubleRow)."""
            key = ("v", br, t)
            if key in emitted:
                return
            emitted.add(key)
            st = sts[br]
            nc.vector.memset(
                st.vaug[t][:].rearrange("p (h m) -> p h m", m=HD + 1)[:, :, HD:HD + 1], WS
            )
            for (coff, cw) in [(0, 512), (512, 256)]:
                pst = ps.tile([P, 512], F32, tag="proj", name="proj", bufs=2)
                for c in range(C2):
                    nc.tensor.matmul(
                        pst[:, :cw],
                        xT8[c][:, :, t * P:(t + 1) * P],
                        wv_sb[br][:, 2 * c:2 * c + 2, coff:coff + cw],
                        start=(c == 0), stop=(c == C2 - 1),
                        perf_mode=DR,
                    )
                h0, nh = coff // HD, cw // HD
                dest = st.vaug[t][:].rearrange("p (h m) -> p h m", m=HD + 1)[:, h0:h0 + nh, 0:HD]
                src = pst[:, :cw].rearrange("p (h m) -> p h m", m=HD)
                if has_bqkv:
                    b_src = st.bv[:, coff:coff + cw].rearrange("p (h m) -> p h m", m=HD)
                    nc.vector.tensor_tensor(dest, src, b_src, op=ALU.add)
                else:
                    nc.vector.tensor_copy(dest, src)

        def outproj_chunk(br, t, ci, cs=(0, 1, 2), first=True):
            """out-projection (c2 blocks `cs`) for chunk ci of own tile t,
            accumulated into ys[t]."""
            key = ("op", br, t, ci, cs)
            if key in emitted:
                return
            emitted.add(key)
            st = sts[br]
            (coff, cw) = [(0, 512), (512, 256)][ci]
            yp = ps.tile([P, 512], F32, tag="proj", name="yp", bufs=2)
            for k, c in enumerate(cs):
                nc.tensor.matmul(
                    yp[:, :cw],
                    st.oT8[:, 2 * c:2 * c + 2, t * P:(t + 1) * P],
                    wp_sb[br][:, 2 * c:2 * c + 2, coff:coff + cw],
                    start=(k == 0), stop=(k == len(cs) - 1),
                    perf_mode=DR,
                )
            dst = ys[t][:, coff:coff + cw]
            if br == 0 and first:
                nc.vector.tensor_tensor(dst, yp[:, :cw], xo_sb[:, t, coff:coff + cw], op=ALU.add)
            else:
                nc.vector.tensor_tensor(dst, dst, yp[:, :cw], op=ALU.add)
            if has_bp and cs[-1] == 2:
                nc.vector.tensor_tensor(dst, dst, bp_b[br][:, coff:coff + cw], op=ALU.add)

        def layernorm_tile(t):
            tsum = stat_pool.tile([P, 1], F32, tag="tsum", name="tsum")
            ycp = xc_pool.tile([P, DIM], F32, tag="ycp", name="ycp")
            nc.scalar.activation(ycp[:], ys[t][:], AF.Copy, accum_out=tsum[:])
            nmu = stat_pool.tile([P, 1], F32, tag="mu", name="nmu")
            nc.scalar.mul(nmu[:], tsum[:], -1.0 / DIM)
            sq = xc_pool.tile([P, DIM], F32, tag="sqt", name="sq")
            ssq = stat_pool.tile([P, 1], F32, tag="ssq", name="ssq")
            nc.scalar.activation(sq[:], ys[t][:], AF.Square, bias=nmu[:], scale=1.0, accum_out=ssq[:])
            std = stat_pool.tile([P, 1], F32, tag="std", name="std")
            nc.scalar.activation(std[:], ssq[:], AF.Sqrt, bias=ebias[:], scale=1.0 / DIM)
            rstd = stat_pool.tile([P, 1], F32, tag="rstd", name="rstd")
            nc.vector.reciprocal(rstd[:], std[:])
            noff = stat_pool.tile([P, 1], F32, tag="noff", name="noff")
            nc.vector.tensor_tensor(noff[:], nmu[:], rstd[:], op=ALU.mult)
            xn = xc_pool.tile([P, DIM], F32, tag="xnt", name="xn")
            nc.vector.tensor_scalar(xn[:], ys[t][:], rstd[:], noff[:], op0=ALU.mult, op1=ALU.add)
            yq = [nc.sync, nc.scalar, nc.gpsimd, nc.sync][t]
            if has_gb:
                xg = xc_pool.tile([P, DIM], F32, tag="xgt", name="xg")
                nc.gpsimd.tensor_tensor(xg[:], xn[:], gamma_b[:], op=ALU.mult)
                yo = xc_pool.tile([P, DIM], F32, tag="yot", name="yo")
                nc.gpsimd.tensor_tensor(yo[:], xg[:], beta_b[:], op=ALU.add)
                yq.dma_start(y_out[t], yo[:])
            else:
                yq.dma_start(y_out[t], xn[:])

        # ---- attention (row-tiled head pairs, grouped EXPs, SW-pipelined) ----
        def attn_gen(br):
            st = sts[br]
            causal = st.causal

            if causal:
                j_iter = [(j, (max(j, OWN_CH0) - OWN_CH0) * P, (TT - max(j, OWN_CH0)) * P)
                          for j in range(TT)]
            else:
                j_iter = [(j, 0, (j - OWN_CH0 + 1) * P)
                          for j in range(TT - 1, OWN_CH0 - 1, -1)]
            nj = len(j_iter)
            # pack pairs of j-chunks contiguously into one 2-bank PSUM tile so
            # a single EXP (per head) covers both -> halves ACT op count
            groups = []
            for i in range(0, nj, 2):
                ch = j_iter[i:i + 2]
                offs, o = [], 0
                for (j, qoff, w) in ch:
                    offs.append(o)
                    o += w
                groups.append([(j, qoff, w, off) for (j, qoff, w), off in zip(ch, offs)])
            G = len(groups)

            def norm_head(ops, row0, m_):
                # reciprocal_approx_fast misreads PSUM on HW -- stage den
                # through SBUF first (DVE copy), then take the reciprocal.
                den = rb_pool.tile([1, 512], F32, tag="den", name="den")
                nc.vector.tensor_copy(den[:], ops[HD:HD + 1, :])
                r = rb_pool.tile([1, 512], F32, tag="r", name="r")
                with nc.allow_low_precision(reason="f32 softmax reciprocal"):
                    nc.vector.reciprocal_approx_fast(r[:], den[:])
                rbt = rb_pool.tile([HD, 512], F32, tag="rb", name="rb")
                nc.gpsimd.partition_broadcast(rbt[:], r[:])
                nc.vector.tensor_tensor(
                    st.oT8[row0:row0 + HD, m_, :], ops[0:HD, :], rbt[:], op=ALU.mult
                )

            for m in range(NP):
                ensure_qk_pair(br, m)
                hA, hB = 2 * m, 2 * m + 1
                oA = ps.tile([HD + 1, 512], F32, tag="oTA", name="oTA", bufs=1)
                oB = ps.tile([HD + 1, 512], F32, tag="oTB", name="oTB", bufs=1)
                exg = {}

                def emit_oT(g):
                    exA, exB, ch = exg.pop(g)
                    for ci, (j, qoff, w, off) in enumerate(ch):
                        v_tile(br, j)
                        start = (g == 0 and ci == 0)
                        stop = (g == G - 1 and ci == len(ch) - 1)
                        nc.tensor.matmul(
                            oA[:, qoff:qoff + w],
                            st.vaug[j][:, hA * (HD + 1):(hA + 1) * (HD + 1)],
                            exA[:, off:off + w], start=start, stop=stop,
                        )
                        nc.tensor.matmul(
                            oB[:, qoff:qoff + w],
                            st.vaug[j][:, hB * (HD + 1):(hB + 1) * (HD + 1)],
                            exB[:, off:off + w], start=start, stop=stop,
                        )

                for g in range(G):
                    ch = groups[g]
                    wtot = sum(c[2] for c in ch)
                    sA = ps.tile([P, 1024], F32, tag="sTA", name="sTA", bufs=1)
                    sB = ps.tile([P, 1024], F32, tag="sTB", name="sTB", bufs=1)
                    for (j, qoff, w, off) in ch:
                        nc.tensor.matmul(
                            sA[:, off:off + w],
                            st.kT[m][0:HD, j * P:(j + 1) * P],
                            st.qT[m][0:HD, qoff:qoff + w],
                        )
                        nc.tensor.matmul(
                            sB[:, off:off + w],
                            st.kT[m][HD:P, j * P:(j + 1) * P],
                            st.qT[m][HD:P, qoff:qoff + w],
                        )
                    exA = expT_pool.tile([P, 1024], F16, tag="exA", name="exA", bufs=3)
                    exB = expT_pool.tile([P, 1024], F16, tag="exB", name="exB", bufs=3)
                    nc.scalar.activation(exA[:, :wtot], sA[:, :wtot], AF.Exp, bias=zbias[:], scale=EXP_SCALE)
                    nc.scalar.activation(exB[:, :wtot], sB[:, :wtot], AF.Exp, bias=zbias[:], scale=EXP_SCALE)
                    for (j, qoff, w, off) in ch:
                        if j >= OWN_CH0:
                            d0 = off + (0 if causal else w - P)
                            # keep k<=q (causal: f-p>=0) / k>=q (anticausal: p-f>=0)
                            cm = -1 if causal else 1
                            pat = [[1, P]] if causal else [[-1, P]]
                            for ex in (exA, exB):
                                nc.gpsimd.affine_select(
                                    out=ex[:, d0:d0 + P], in_=ex[:, d0:d0 + P],
                                    compare_op=ALU.is_ge, fill=0.0,
                                    base=0, pattern=pat, channel_multiplier=cm,
                                )
                    exg[g] = (exA, exB, ch)
                    if g >= 1:
                        emit_oT(g - 1)
                    yield
                emit_oT(G - 1)
                norm_head(oA, 0, m)
                norm_head(oB, HD, m)
                yield

        # ---- emission driver ----
        fillers = []
        for m in range(1, NP):
            fillers.append(("qk", 0, m))
            fillers.append(("qk", 0, CB + m))
        for m in range(NP):
            fillers.append(("qk", 1, m))
            fillers.append(("qk", 1, CB + m))
        for t in range(OWN_CH0, TT):
            fillers.append(("v", 1, t))

        def emit_filler():
            while fillers:
                key = fillers.pop(0)
                if key in emitted:
                    continue
                if key[0] == "qk":
                    qk_nblock(key[1], key[2])
                elif key[0] == "v":
                    v_tile(key[1], key[2])
                elif key[0] == "op":
                    outproj_chunk(key[1], key[2], key[3], key[4])
                return

        for _ in attn_gen(0):
            emit_filler()
        for t in range(OT):
            fillers.append(("op", 0, t, 0, (0, 1, 2)))
            fillers.append(("op", 0, t, 1, (0, 1, 2)))
        # b1 attention: 6 pairs x (G+1)=3 yields; pair-3 norm lands at yield 12
        yc = 0
        for _ in attn_gen(1):
            yc += 1
            if yc == 13:
                for t in range(OT):
                    fillers.append(("op", 1, t, 0, (0, 1)))
                    fillers.append(("op", 1, t, 1, (0, 1)))
            emit_filler()
            emit_filler()
        while fillers:
            emit_filler()
        for t in range(OT):
            outproj_chunk(1, t, 0, cs=(2,), first=False)
            outproj_chunk(1, t, 1, cs=(2,), first=False)
            layernorm_tile(t)

    nc.compile()
    return nc


_CACHE = {}


def _get_program(has_bqkv, has_bp, has_gb):
    key = (has_bqkv, has_bp, has_gb)
    if key not in _CACHE:
        _CACHE[key] = build_program(has_bqkv, has_bp, has_gb)
    return _CACHE[key]


def _pack_qk(W):
    """[768, 2304] -> packed q/k stationary tiles [128, 12, 6, 128] (fp8, x16)."""
    return _f8(WS * W[:, :2 * DIM].reshape(CB, P, 2 * CB, P).transpose(1, 2, 0, 3))


def _pack_mov(W):
    """[768, N] -> moving operand [128, 6, N] (fp8, x16)."""
    n = W.shape[1]
    return _f8(WS * W.reshape(CB, P, n).transpose(1, 0, 2))


def _pack_xT(xb):
    """[T, 768] -> [128, 6, T] fp8."""
    return _f8(xb.T.reshape(CB, P, T).transpose(1, 0, 2))


def make_in_maps(x, Wqkv_c, bqkv_c, Wp_c, bp_c, Wqkv_ac, bqkv_ac, Wp_ac, bp_ac, gamma, beta):
    """Build the 8 per-core input maps (batch-major, half-minor)."""
    qk_c, qk_ac = _pack_qk(Wqkv_c), _pack_qk(Wqkv_ac)
    wv_c = _pack_mov(Wqkv_c[:, 2 * DIM:])
    wv_ac = _pack_mov(Wqkv_ac[:, 2 * DIM:])
    wp_c8, wp_ac8 = _pack_mov(Wp_c), _pack_mov(Wp_ac)
    b_c, b_ac = _f32(WS * bqkv_c), _f32(WS * bqkv_ac)
    bp_c16, bp_ac16 = _f32(WS * bp_c), _f32(WS * bp_ac)
    in_maps = []
    for b in range(B):
        for half in (0, 1):
            if half == 1:
                xb = x[b]
                Ws = (qk_c, wv_c, wp_c8, b_c, bp_c16, qk_ac, wv_ac, wp_ac8, b_ac, bp_ac16)
            else:
                xb = x[b][::-1]
                Ws = (qk_ac, wv_ac, wp_ac8, b_ac, bp_ac16, qk_c, wv_c, wp_c8, b_c, bp_c16)
            in_maps.append({
                "xT8": _pack_xT(xb),
                "x_own": _f16(WS * xb[OWN:]),
                "wqk1": Ws[0], "wv1": Ws[1], "wp1": Ws[2], "b1": Ws[3], "bp1": Ws[4],
                "wqk2": Ws[5], "wv2": Ws[6], "wp2": Ws[7], "b2": Ws[8], "bp2": Ws[9],
                "gamma": gamma, "beta": beta,
            })
    return in_maps


def assemble_output(results):
    out = np.empty((B, T, DIM), dtype=np.float32)
    for b in range(B):
        for half in (0, 1):
            yc = results[b * 2 + half]["y"]
            if half == 1:
                out[b, OWN:] = yc
            else:
                out[b, :OWN] = yc[::-1]
    return out


def kernel(x, Wqkv_c, bqkv_c, Wp_c, bp_c, Wqkv_ac, bqkv_ac, Wp_ac, bp_ac, gamma, beta):
    x = _f32(x)
    Wqkv_c, Wp_c, Wqkv_ac, Wp_ac = map(_f32, (Wqkv_c, Wp_c, Wqkv_ac, Wp_ac))
    bqkv_c, bp_c, bqkv_ac, bp_ac = map(_f32, (bqkv_c, bp_c, bqkv_ac, bp_ac))
    gamma, beta = map(_f32, (gamma, beta))

    has_bqkv = bool(np.any(bqkv_c) or np.any(bqkv_ac))
    has_bp = bool(np.any(bp_c) or np.any(bp_ac))
    has_gb = not (np.all(gamma == 1.0) and np.all(beta == 0.0))
    nc = _get_program(has_bqkv, has_bp, has_gb)

    in_maps = make_in_maps(x, Wqkv_c, bqkv_c, Wp_c, bp_c,
                           Wqkv_ac, bqkv_ac, Wp_ac, bp_ac, gamma, beta)
    res = bass_utils.run_bass_kernel_spmd(nc, in_maps, core_ids=list(range(8)))
    return assemble_output(res.results)


# revision 12
# speedup vs baseline: 1.1069x; 1.1069x over previous
"""Trainium2 Bass kernel for dual-branch (causal + anticausal) attention + residual + LayerNorm.

Reference computation (per batch b):
  out_c  = causal_attn(x_b; Wqkv_c, Wp_c)      (mask j <= i)
  out_ac = anticausal_attn(x_b; Wqkv_ac, Wp_ac) (mask j >= i)
  y = LayerNorm(x + out_c + out_ac) * gamma + beta

Sharding: 8 cores = 4 batches x 2 sequence-halves. Each core computes BOTH
branches for its 512 own tokens (recomputing k/v projections for the needed
part of the 1024-token sequence locally -> zero cross-core communication).
A single SPMD program always "owns" the SECOND half of the sequence; cores
responsible for the first half receive the token-REVERSED sequence with the
causal/anticausal weights swapped, and their output rows are un-reversed on
the host.

v2 performance structure:
 - All projection matmuls (q/k, v, out) run in fp8e4m3 with
   perf_mode=DoubleRow (K=256 per matmul -> half the PE stream time).
   Weights are host-scaled by 16 to dodge fp8 subnormals; the 16x factor
   rides through the residual sum and cancels in LayerNorm (scale-invariant).
 - Attention scores run as row-tiled head PAIRS (two K=64 matmuls
   concurrently in the PE array, partitions 0:64 / 64:128).
 - Softmax denominator comes from an appended ones(=16)-column on V;
   diagonal-block masking is done in-place on the exp tiles by GpSimd
   affine_select (no mask tensors, no DVE multiplies).
 - Emission order interleaves branch-1 projections (PE-dense, Scalar-free)
   into branch-0's attention (Scalar-bound), and branch-0's out-projection
   into branch-1's attention, so the PE never starves while ACT does EXPs.
"""

import numpy as np
from contextlib import ExitStack

import concourse.bass as bass
import concourse.tile as tile
import concourse.mybir as mybir
from concourse import bacc
from concourse import bass_utils

F32 = mybir.dt.float32
F16 = mybir.dt.float16
F8 = mybir.dt.float8e4
AF = mybir.ActivationFunctionType
ALU = mybir.AluOpType
DR = mybir.MatmulPerfMode.DoubleRow

DIM = 768
HEADS = 12
HD = 64
T = 1024
OWN = 512
B = 4
EPS = 1e-5
P = 128
CB = DIM // P          # 6 contraction blocks
C2 = CB // 2           # 3 DoubleRow contraction super-blocks (256 each)
TT = T // P            # 8 token tiles (full sequence)
OT = OWN // P          # 4 own token tiles
OWN_CH0 = TT - OT      # own q-chunks are global chunks 4..7
NP = HEADS // 2        # 6 head pairs
WS = 16.0              # host-side weight scale (fp8 subnormal dodge)
EXP_SCALE = 1.0 / (8.0 * WS * WS)   # 1/sqrt(HD) / WS^2

F8NP = mybir.dt.np(F8)


def _f32(x):
    return np.ascontiguousarray(np.asarray(x, dtype=np.float32))


def _f16(x):
    return np.ascontiguousarray(np.asarray(x, dtype=np.float16))


def _f8(x):
    return np.ascontiguousarray(np.asarray(x, dtype=np.float32).astype(F8NP))


def build_program(has_bqkv: bool, has_bp: bool, has_gb: bool):
    nc = bacc.Bacc("TRN2", target_bir_lowering=False)

    xT_d = nc.dram_tensor("xT8", [P, CB, T], F8, kind="ExternalInput")
    xo_d = nc.dram_tensor("x_own", [OWN, DIM], F16, kind="ExternalInput")
    wqk_d = [nc.dram_tensor(f"wqk{i}", [2 * CB, P, CB, P], F8, kind="ExternalInput") for i in (1, 2)]
    wv_d = [nc.dram_tensor(f"wv{i}", [P, CB, DIM], F8, kind="ExternalInput") for i in (1, 2)]
    wp_d = [nc.dram_tensor(f"wp{i}", [P, CB, DIM], F8, kind="ExternalInput") for i in (1, 2)]
    b_d = [nc.dram_tensor(f"b{i}", [3 * DIM], F32, kind="ExternalInput") for i in (1, 2)]
    bp_d = [nc.dram_tensor(f"bp{i}", [DIM], F32, kind="ExternalInput") for i in (1, 2)]
    gamma_d = nc.dram_tensor("gamma", [DIM], F32, kind="ExternalInput")
    beta_d = nc.dram_tensor("beta", [DIM], F32, kind="ExternalInput")
    y_d = nc.dram_tensor("y", [OWN, DIM], F32, kind="ExternalOutput")

    with tile.TileContext(nc) as tc, ExitStack() as ctx:
        const = ctx.enter_context(tc.tile_pool(name="const", bufs=1))
        wqk_pool = ctx.enter_context(tc.tile_pool(name="wqk", bufs=3))
        persist = ctx.enter_context(tc.tile_pool(name="persist", bufs=1))
        wvp_pool = ctx.enter_context(tc.tile_pool(name="wvp", bufs=1))
        expT_pool = ctx.enter_context(tc.tile_pool(name="expT", bufs=1))
        rb_pool = ctx.enter_context(tc.tile_pool(name="rb", bufs=3))
        stat_pool = ctx.enter_context(tc.tile_pool(name="stat", bufs=8))
        xc_pool = ctx.enter_context(tc.tile_pool(name="xc", bufs=2))
        yacc_pool = ctx.enter_context(tc.tile_pool(name="yacc", bufs=1))
        ps = ctx.enter_context(tc.tile_pool(name="ps", bufs=1, space="PSUM"))

        # ---- constants / full-kernel-lifetime tensors ----
        # xT in fp8, split into 3 DoubleRow contraction super-blocks so the
        # first projection only waits on one third of the input DMA.
        xT8 = [const.tile([P, 2, T], F8, tag=f"xT{c}", name=f"xT{c}") for c in range(C2)]
        # own-half columns first (the q-projection needs only those), spread
        # across the scalar+gpsimd queues; sync is reserved for the wqk stream
        nc.scalar.dma_start(xT8[0][:, :, OWN:T], xT_d[:, 0:2, OWN:T])
        nc.gpsimd.dma_start(xT8[1][:, :, OWN:T], xT_d[:, 2:4, OWN:T])
        nc.scalar.dma_start(xT8[2][:, :, OWN:T], xT_d[:, 4:6, OWN:T])
        nc.gpsimd.dma_start(xT8[0][:, :, 0:OWN], xT_d[:, 0:2, 0:OWN])
        nc.scalar.dma_start(xT8[1][:, :, 0:OWN], xT_d[:, 2:4, 0:OWN])
        nc.gpsimd.dma_start(xT8[2][:, :, 0:OWN], xT_d[:, 4:6, 0:OWN])

        xo_sb = const.tile([P, OT, DIM], F16)
        nc.gpsimd.dma_start(xo_sb[:], xo_d.rearrange("(tb p) c -> p tb c", p=P))

        gamma_b = beta_b = None
        if has_gb:
            gamma_b = const.tile([P, DIM], F32)
            nc.gpsimd.dma_start(gamma_b[:], bass.AP(tensor=gamma_d, offset=0, ap=[[0, P], [1, DIM]]))
            beta_b = const.tile([P, DIM], F32)
            nc.gpsimd.dma_start(beta_b[:], bass.AP(tensor=beta_d, offset=0, ap=[[0, P], [1, DIM]]))

        zbias = const.tile([P, 1], F32)
        nc.vector.memset(zbias[:], 0.0)
        ebias = const.tile([P, 1], F32)
        nc.vector.memset(ebias[:], EPS)

        # warm the GpSimd ucode libraries (affine_select + partition
        # broadcast) while the input DMAs run: the first cold LIBRARY_RELOAD
        # mid-kernel costs ~6us of all-engine stall otherwise
        warm = const.tile([4, 16], F32)
        warm2 = const.tile([4, 16], F32)
        nc.gpsimd.memset(warm[:], 1.0)
        nc.gpsimd.affine_select(
            out=warm[:], in_=warm[:], compare_op=ALU.is_ge, fill=0.0,
            base=0, pattern=[[-1, 16]], channel_multiplier=1,
        )
        nc.gpsimd.partition_broadcast(warm2[:], warm[0:1, :])
        nc.gpsimd.affine_select(
            out=warm[:], in_=warm[:], compare_op=ALU.is_ge, fill=0.0,
            base=0, pattern=[[1, 16]], channel_multiplier=-1,
        )
        nc.gpsimd.partition_broadcast(warm2[:], warm[0:1, :])

        bp_b = None
        if has_bp:
            bp_b = [const.tile([P, DIM], F32, tag=f"bp_b{i}", name=f"bp_b{i}") for i in range(2)]
            for i in range(2):
                nc.gpsimd.dma_start(bp_b[i][:], bass.AP(tensor=bp_d[i], offset=0, ap=[[0, P], [1, DIM]]))

        # moving-weight tensors (fp8, resident per branch)
        wv_sb = [wvp_pool.tile([P, CB, DIM], F8, tag="wv", name=f"wv{br}", bufs=2) for br in range(2)]
        wp_sb = [wvp_pool.tile([P, CB, DIM], F8, tag="wp", name=f"wp{br}", bufs=2) for br in range(2)]
        nc.scalar.dma_start(wv_sb[0][:], wv_d[0][:])
        nc.gpsimd.dma_start(wv_sb[1][:], wv_d[1][:])
        nc.scalar.dma_start(wp_sb[0][:], wp_d[0][:])
        nc.gpsimd.dma_start(wp_sb[1][:], wp_d[1][:])

        # y accumulator tiles (live across both branches)
        ys = [yacc_pool.tile([P, DIM], F32, tag=f"ys{t}", name=f"ys{t}") for t in range(OT)]
        y_out = y_d.rearrange("(tb p) c -> tb p c", p=P)

        # ---- per-branch state ----
        class St:
            pass

        sts = []
        for br in range(2):
            st = St()
            st.br = br
            st.causal = br == 0
            st.full_kv = st.causal
            st.kT = [persist.tile([P, T], F16, tag=f"kT{i}", name=f"kT{i}_{br}", bufs=2) for i in range(CB)]
            st.qT = [persist.tile([P, OWN], F16, tag=f"qT{i}", name=f"qT{i}_{br}", bufs=2) for i in range(CB)]
            st.vaug = [persist.tile([P, HEADS * (HD + 1)], F16, tag=f"va{t}", name=f"va{t}_{br}", bufs=2) for t in range(TT)]
            st.oT8 = persist.tile([P, CB, OWN], F8, tag="oT8", name=f"oT8_{br}", bufs=2)
            st.bqk = None
            st.bv = None
            if has_bqkv:
                st.bqk = persist.tile([P, 2 * CB], F32, tag="bqk", bufs=2)
                nc.sync.dma_start(st.bqk[:], b_d[br][0:2 * DIM].rearrange("(n p) -> p n", p=P))
                st.bv = persist.tile([P, DIM], F32, tag="bv", bufs=2)
                nc.sync.dma_start(st.bv[:], bass.AP(tensor=b_d[br], offset=2 * DIM, ap=[[0, P], [1, DIM]]))
            sts.append(st)

        emitted = set()

        # ---- projection steps ----
        def qk_nblock(br, n):
            """q/k projection for one 128-wide output block n (fp8 DoubleRow)."""
            key = ("qk", br, n)
            if key in emitted:
                return
            emitted.add(key)
            st = sts[br]
            is_q = n < CB
            if is_q:
                chunks = [(OWN, OWN)]
            elif st.full_kv:
                chunks = [(0, 512), (512, 512)]
            else:
                chunks = [(512, 512)]
            wt = wqk_pool.tile([P, CB, P], F8)
            nc.sync.dma_start(wt[:], wqk_d[br][n])
            for (t0, tw) in chunks:
                pst = ps.tile([P, 512], F32, tag="proj", name="proj", bufs=2)
                for c in range(C2):
                    nc.tensor.matmul(
                        pst[:, :tw],
                        wt[:, 2 * c:2 * c + 2, :],
                        xT8[c][:, :, t0:t0 + tw],
                        start=(c == 0), stop=(c == C2 - 1),
                        perf_mode=DR,
                    )
                if is_q:
                    dest = st.qT[n][:, :]
                else:
                    dest = st.kT[n - CB][:, t0:t0 + tw]
                if has_bqkv:
                    nc.vector.tensor_scalar_add(dest, pst[:, :tw], st.bqk[:, n:n + 1])
                else:
                    nc.vector.tensor_copy(dest, pst[:, :tw])

        def ensure_qk_pair(br, m):
            qk_nblock(br, m)
            qk_nblock(br, CB + m)

        def v_tile(br, t):
            """v projection for token tile t -> vaug[t] (fp8 DoubleRow)."""
            key = ("v", br, t)
            if key in emitted:
                return
            emitted.add(key)
            st = sts[br]
            nc.vector.memset(
                st.vaug[t][:].rearrange("p (h m) -> p h m", m=HD + 1)[:, :, HD:HD + 1], WS
            )
            for (coff, cw) in [(0, 512), (512, 256)]:
                pst = ps.tile([P, 512], F32, tag="proj", name="proj", bufs=2)
                for c in range(C2):
                    nc.tensor.matmul(
                        pst[:, :cw],
                        xT8[c][:, :, t * P:(t + 1) * P],
                        wv_sb[br][:, 2 * c:2 * c + 2, coff:coff + cw],
                        start=(c == 0), stop=(c == C2 - 1),
                        perf_mode=DR,
                    )
                h0, nh = coff // HD, cw // HD
                dest = st.vaug[t][:].rearrange("p (h m) -> p h m", m=HD + 1)[:, h0:h0 + nh, 0:HD]
                src = pst[:, :cw].rearrange("p (h m) -> p h m", m=HD)
                if has_bqkv:
                    b_src = st.bv[:, coff:coff + cw].rearrange("p (h m) -> p h m", m=HD)
                    nc.vector.tensor_tensor(dest, src, b_src, op=ALU.add)
                else:
                    nc.vector.tensor_copy(dest, src)

        def outproj_chunk(br, t, ci, cs=(0, 1, 2), first=True):
            """out-projection (c2 blocks `cs`) for chunk ci of own tile t,
            accumulated into ys[t]."""
            key = ("op", br, t, ci, cs)
            if key in emitted:
                return
            emitted.add(key)
            st = sts[br]
            (coff, cw) = [(0, 512), (512, 256)][ci]
            yp = ps.tile([P, 512], F32, tag="proj", name="yp", bufs=2)
            for k, c in enumerate(cs):
                nc.tensor.matmul(
                    yp[:, :cw],
                    st.oT8[:, 2 * c:2 * c + 2, t * P:(t + 1) * P],
                    wp_sb[br][:, 2 * c:2 * c + 2, coff:coff + cw],
                    start=(k == 0), stop=(k == len(cs) - 1),
                    perf_mode=DR,
                )
            dst = ys[t][:, coff:coff + cw]
            if br == 0 and first:
                nc.vector.tensor_tensor(dst, yp[:, :cw], xo_sb[:, t, coff:coff + cw], op=ALU.add)
            else:
                nc.vector.tensor_tensor(dst, dst, yp[:, :cw], op=ALU.add)
            if has_bp and cs[-1] == 2:
                nc.vector.tensor_tensor(dst, dst, bp_b[br][:, coff:coff + cw], op=ALU.add)

        def layernorm_tile(t):
            tsum = stat_pool.tile([P, 1], F32, tag="tsum", name="tsum")
            ycp = xc_pool.tile([P, DIM], F32, tag="ycp", name="ycp")
            nc.scalar.activation(ycp[:], ys[t][:], AF.Copy, accum_out=tsum[:])
            nmu = stat_pool.tile([P, 1], F32, tag="mu", name="nmu")
            nc.scalar.mul(nmu[:], tsum[:], -1.0 / DIM)
            sq = xc_pool.tile([P, DIM], F32, tag="sqt", name="sq")
            ssq = stat_pool.tile([P, 1], F32, tag="ssq", name="ssq")
            nc.scalar.activation(sq[:], ys[t][:], AF.Square, bias=nmu[:], scale=1.0, accum_out=ssq[:])
            std = stat_pool.tile([P, 1], F32, tag="std", name="std")
            nc.scalar.activation(std[:], ssq[:], AF.Sqrt, bias=ebias[:], scale=1.0 / DIM)
            rstd = stat_pool.tile([P, 1], F32, tag="rstd", name="rstd")
            nc.vector.reciprocal(rstd[:], std[:])
            noff = stat_pool.tile([P, 1], F32, tag="noff", name="noff")
            nc.vector.tensor_tensor(noff[:], nmu[:], rstd[:], op=ALU.mult)
            xn = xc_pool.tile([P, DIM], F32, tag="xnt", name="xn")
            nc.vector.tensor_scalar(xn[:], ys[t][:], rstd[:], noff[:], op0=ALU.mult, op1=ALU.add)
            yq = [nc.sync, nc.scalar, nc.gpsimd, nc.sync][t]
            if has_gb:
                xg = xc_pool.tile([P, DIM], F32, tag="xgt", name="xg")
                nc.gpsimd.tensor_tensor(xg[:], xn[:], gamma_b[:], op=ALU.mult)
                yo = xc_pool.tile([P, DIM], F32, tag="yot", name="yo")
                nc.gpsimd.tensor_tensor(yo[:], xg[:], beta_b[:], op=ALU.add)
                yq.dma_start(y_out[t], yo[:])
            else:
                yq.dma_start(y_out[t], xn[:])

        # ---- attention (row-tiled head pairs, grouped EXPs, SW-pipelined) ----
        def attn_gen(br):
            st = sts[br]
            causal = st.causal

            if causal:
                j_iter = [(j, (max(j, OWN_CH0) - OWN_CH0) * P, (TT - max(j, OWN_CH0)) * P)
                          for j in range(TT)]
            else:
                j_iter = [(j, 0, (j - OWN_CH0 + 1) * P)
                          for j in range(TT - 1, OWN_CH0 - 1, -1)]
            nj = len(j_iter)
            # pack pairs of j-chunks contiguously into one 2-bank PSUM tile so
            # a single EXP (per head) covers both -> halves ACT op count
            groups = []
            for i in range(0, nj, 2):
                ch = j_iter[i:i + 2]
                offs, o = [], 0
                for (j, qoff, w) in ch:
                    offs.append(o)
                    o += w
                groups.append([(j, qoff, w, off) for (j, qoff, w), off in zip(ch, offs)])
            G = len(groups)

            def norm_head(ops, row0, m_):
                # reciprocal_approx_fast misreads PSUM on HW -- stage den
                # through SBUF first (DVE copy), then take the reciprocal.
                den = rb_pool.tile([1, 512], F32, tag="den", name="den")
                nc.vector.tensor_copy(den[:], ops[HD:HD + 1, :])
                r = rb_pool.tile([1, 512], F32, tag="r", name="r")
                with nc.allow_low_precision(reason="f32 softmax reciprocal"):
                    nc.vector.reciprocal_approx_fast(r[:], den[:])
                rbt = rb_pool.tile([HD, 512], F32, tag="rb", name="rb")
                nc.gpsimd.partition_broadcast(rbt[:], r[:])
                nc.vector.tensor_tensor(
                    st.oT8[row0:row0 + HD, m_, :], ops[0:HD, :], rbt[:], op=ALU.mult
                )

            for m in range(NP):
                ensure_qk_pair(br, m)
                hA, hB = 2 * m, 2 * m + 1
                oA = ps.tile([HD + 1, 512], F32, tag="oTA", name="oTA", bufs=1)
                oB = ps.tile([HD + 1, 512], F32, tag="oTB", name="oTB", bufs=1)
                exg = {}

                def emit_oT(g):
                    exA, exB, ch = exg.pop(g)
                    for ci, (j, qoff, w, off) in enumerate(ch):
                        v_tile(br, j)
                        start = (g == 0 and ci == 0)
                        stop = (g == G - 1 and ci == len(ch) - 1)
                        nc.tensor.matmul(
                            oA[:, qoff:qoff + w],
                            st.vaug[j][:, hA * (HD + 1):(hA + 1) * (HD + 1)],
                            exA[:, off:off + w], start=start, stop=stop,
                        )
                        nc.tensor.matmul(
                            oB[:, qoff:qoff + w],
                            st.vaug[j][:, hB * (HD + 1):(hB + 1) * (HD + 1)],
                            exB[:, off:off + w], start=start, stop=stop,
                        )

                for g in range(G):
                    ch = groups[g]
                    wtot = sum(c[2] for c in ch)
                    sA = ps.tile([P, 1024], F32, tag="sTA", name="sTA", bufs=1)
                    sB = ps.tile([P, 1024], F32, tag="sTB", name="sTB", bufs=1)
                    for (j, qoff, w, off) in ch:
                        nc.tensor.matmul(
                            sA[:, off:off + w],
                            st.kT[m][0:HD, j * P:(j + 1) * P],
                            st.qT[m][0:HD, qoff:qoff + w],
                        )
                        nc.tensor.matmul(
                            sB[:, off:off + w],
                            st.kT[m][HD:P, j * P:(j + 1) * P],
                            st.qT[m][HD:P, qoff:qoff + w],
                        )
                    exA = expT_pool.tile([P, 1024], F16, tag="exA", name="exA", bufs=3)
                    exB = expT_pool.tile([P, 1024], F16, tag="exB", name="exB", bufs=3)
                    nc.scalar.activation(exA[:, :wtot], sA[:, :wtot], AF.Exp, bias=zbias[:], scale=EXP_SCALE)
                    nc.scalar.activation(exB[:, :wtot], sB[:, :wtot], AF.Exp, bias=zbias[:], scale=EXP_SCALE)
                    for (j, qoff, w, off) in ch:
                        if j >= OWN_CH0:
                            d0 = off + (0 if causal else w - P)
                            # keep k<=q (causal: f-p>=0) / k>=q (anticausal: p-f>=0)
                            cm = -1 if causal else 1
                            pat = [[1, P]] if causal else [[-1, P]]
                            for ex in (exA, exB):
                                nc.gpsimd.affine_select(
                                    out=ex[:, d0:d0 + P], in_=ex[:, d0:d0 + P],
                                    compare_op=ALU.is_ge, fill=0.0,
                                    base=0, pattern=pat, channel_multiplier=cm,
                                )
                    exg[g] = (exA, exB, ch)
                    if g >= 1:
                        emit_oT(g - 1)
                    yield
                emit_oT(G - 1)
                norm_head(oA, 0, m)
                norm_head(oB, HD, m)
                yield

        # ---- emission driver ----
        fillers = []
        for m in range(1, NP):
            fillers.append(("qk", 0, m))
            fillers.append(("qk", 0, CB + m))
        for m in range(NP):
            fillers.append(("qk", 1, m))
            fillers.append(("qk", 1, CB + m))
        for t in range(OWN_CH0, TT):
            fillers.append(("v", 1, t))

        def emit_filler():
            while fillers:
                key = fillers.pop(0)
                if key in emitted:
                    continue
                if key[0] == "qk":
                    qk_nblock(key[1], key[2])
                elif key[0] == "v":
                    v_tile(key[1], key[2])
                elif key[0] == "op":
                    outproj_chunk(key[1], key[2], key[3], key[4])
                return

        for _ in attn_gen(0):
            emit_filler()
        for t in range(OT):
            fillers.append(("op", 0, t, 0, (0, 1, 2)))
            fillers.append(("op", 0, t, 1, (0, 1, 2)))
        # b1 attention: 6 pairs x (G+1)=3 yields; pair-3 norm lands at yield 12
        yc = 0
        for _ in attn_gen(1):
            yc += 1
            if yc == 13:
                for t in range(OT):
                    fillers.append(("op", 1, t, 0, (0, 1)))
                    fillers.append(("op", 1, t, 1, (0, 1)))
            emit_filler()
            emit_filler()
        while fillers:
            emit_filler()
        for t in range(OT):
            outproj_chunk(1, t, 0, cs=(2,), first=False)
            outproj_chunk(1, t, 1, cs=(2,), first=False)
            layernorm_tile(t)

    nc.compile()
    return nc


_CACHE = {}


def _get_program(has_bqkv, has_bp, has_gb):
    key = (has_bqkv, has_bp, has_gb)
    if key not in _CACHE:
        _CACHE[key] = build_program(has_bqkv, has_bp, has_gb)
    return _CACHE[key]


def _pack_qk(W):
    """[768, 2304] -> packed q/k stationary tiles [12, 128, 6, 128] (fp8, x16)."""
    return _f8(WS * W[:, :2 * DIM].reshape(CB, P, 2 * CB, P).transpose(2, 1, 0, 3))


def _pack_mov(W):
    """[768, N] -> moving operand [128, 6, N] (fp8, x16)."""
    n = W.shape[1]
    return _f8(WS * W.reshape(CB, P, n).transpose(1, 0, 2))


def _pack_xT(xb):
    """[T, 768] -> [128, 6, T] fp8."""
    return _f8(xb.T.reshape(CB, P, T).transpose(1, 0, 2))


def make_in_maps(x, Wqkv_c, bqkv_c, Wp_c, bp_c, Wqkv_ac, bqkv_ac, Wp_ac, bp_ac, gamma, beta):
    """Build the 8 per-core input maps (batch-major, half-minor)."""
    qk_c, qk_ac = _pack_qk(Wqkv_c), _pack_qk(Wqkv_ac)
    wv_c = _pack_mov(Wqkv_c[:, 2 * DIM:])
    wv_ac = _pack_mov(Wqkv_ac[:, 2 * DIM:])
    wp_c8, wp_ac8 = _pack_mov(Wp_c), _pack_mov(Wp_ac)
    b_c, b_ac = _f32(WS * bqkv_c), _f32(WS * bqkv_ac)
    bp_c16, bp_ac16 = _f32(WS * bp_c), _f32(WS * bp_ac)
    in_maps = []
    for b in range(B):
        for half in (0, 1):
            if half == 1:
                xb = x[b]
                Ws = (qk_c, wv_c, wp_c8, b_c, bp_c16, qk_ac, wv_ac, wp_ac8, b_ac, bp_ac16)
            else:
                xb = x[b][::-1]
                Ws = (qk_ac, wv_ac, wp_ac8, b_ac, bp_ac16, qk_c, wv_c, wp_c8, b_c, bp_c16)
            in_maps.append({
                "xT8": _pack_xT(xb),
                "x_own": _f16(WS * xb[OWN:]),
                "wqk1": Ws[0], "wv1": Ws[1], "wp1": Ws[2], "b1": Ws[3], "bp1": Ws[4],
                "wqk2": Ws[5], "wv2": Ws[6], "wp2": Ws[7], "b2": Ws[8], "bp2": Ws[9],
                "gamma": gamma, "beta": beta,
            })
    return in_maps


def assemble_output(results):
    out = np.empty((B, T, DIM), dtype=np.float32)
    for b in range(B):
        for half in (0, 1):
            yc = results[b * 2 + half]["y"]
            if half == 1:
                out[b, OWN:] = yc
            else:
                out[b, :OWN] = yc[::-1]
    return out


def kernel(x, Wqkv_c, bqkv_c, Wp_c, bp_c, Wqkv_ac, bqkv_ac, Wp_ac, bp_ac, gamma, beta):
    x = _f32(x)
    Wqkv_c, Wp_c, Wqkv_ac, Wp_ac = map(_f32, (Wqkv_c, Wp_c, Wqkv_ac, Wp_ac))
    bqkv_c, bp_c, bqkv_ac, bp_ac = map(_f32, (bqkv_c, bp_c, bqkv_ac, bp_ac))
    gamma, beta = map(_f32, (gamma, beta))

    has_bqkv = bool(np.any(bqkv_c) or np.any(bqkv_ac))
    has_bp = bool(np.any(bp_c) or np.any(bp_ac))
    has_gb = not (np.all(gamma == 1.0) and np.all(beta == 0.0))
    nc = _get_program(has_bqkv, has_bp, has_gb)

    in_maps = make_in_maps(x, Wqkv_c, bqkv_c, Wp_c, bp_c,
                           Wqkv_ac, bqkv_ac, Wp_ac, bp_ac, gamma, beta)
    res = bass_utils.run_bass_kernel_spmd(nc, in_maps, core_ids=list(range(8)))
    return assemble_output(res.results)
